# revision 1
# baseline (speedup 1.0000x reference)
"""Trainium2 Bass kernel for ComplexAttention.

Math (per (b,t) pair):
    cur2 = [cur_r, cur_i]                       # [2D]
    Q    = cur2 @ qW + qb                       # [D]
    K_s  = H_s @ kW + kb ; V_s = H_s @ vW + vb  # H = [hist_r, hist_i]  [S, 2D]
    sc_s = (Q . K_s) * scale * conf
    w    = softmax(sc) ; ctx = sum_s w_s V_s
    out  = cur + 0.1 * ctx (complex)

Rewrites used (exact):
    Q . K_s = (cur2 @ (qW kW^T) + qb kW^T) . H_s  +  (Q . kb)
        - the (Q . kb) term is constant over s -> softmax-invariant -> dropped.
    ctx = (sum_s w_s H_s) @ vW + vb      (since sum_s w_s = 1)

So per pair only two small contractions touch H (scores dot and the weighted
sum), and the heavy lifting is three big batched matmuls folded into two:
    Qk  = cur2 @ Wqk + bqk      (Wqk = qW kW^T, bqk = qb kW^T, host-folded)
    ctx = hbar @ vW + vb

Sharding: data-parallel over the 4096 (b,t) pairs, 512 per core, weights
replicated.  Pairs are processed in groups of 4 (stacked 4x32 on the 128 SBUF
partitions), 8 groups per H DMA sub-batch, 32 groups per 128-pair batch.
"""

import os
import sys

import numpy as np

os.environ.setdefault("MYCRO_LOCAL_CACHE", "1")

try:
    import concourse.bass as bass
except ImportError:  # pragma: no cover
    sys.path.insert(0, "/opt/trn_rl_repo")
    import concourse.bass as bass

import concourse.mybir as mybir
import concourse.tile as tile
from concourse import bacc
from concourse.bass_utils import run_bass_kernel_spmd

F32 = mybir.dt.float32
F32R = mybir.dt.float32r
F16 = mybir.dt.float16
AX = mybir.AluOpType
AF = mybir.ActivationFunctionType

B, T, S, D = 4, 1024, 32, 512
D2 = 2 * D  # 1024, concat(real, imag) feature dim
E = 2 * D   # 1024, history feature dim
N_CORES = 8
PAIRS = B * T
SCALE = float(D) ** -0.5


def r(ap):
    return ap.bitcast(F32R)


DEBUG_STOP = os.environ.get("K_DEBUG_STOP", "")  # "", "A", "B", "NOHB", "NOCTX"
BENCH_LOOP = int(os.environ.get("K_BENCH_LOOP", "0"))  # >0: repeat body N times
NO_TTR = os.environ.get("K_NO_TTR", "") == "1"   # replace fused dot with memset
NO_HDMA = os.environ.get("K_NO_HDMA", "") == "1"  # memset H instead of DMA load


def build(ppc: int) -> bass.Bass:
    """Build the per-core SPMD program for `ppc` pairs per core."""
    assert ppc % 128 == 0
    nb = ppc // 128      # batches of 128 pairs
    ng = ppc // 4        # groups of 4 pairs (core total)

    nc = bacc.Bacc("TRN2", target_bir_lowering=False)

    hist_r = nc.declare_dram_parameter("hist_real", [ppc, S, D], F32, isOutput=False)
    hist_i = nc.declare_dram_parameter("hist_imag", [ppc, S, D], F32, isOutput=False)
    cur_r = nc.declare_dram_parameter("cur_r", [ppc, D], F32, isOutput=False)
    cur_i = nc.declare_dram_parameter("cur_i", [ppc, D], F32, isOutput=False)
    cur2t = nc.declare_dram_parameter("cur2t", [D2, ppc], F32, isOutput=False)
    wqk = nc.declare_dram_parameter("wqk", [D2, E], F32, isOutput=False)
    bqk = nc.declare_dram_parameter("bqk", [1, E], F32, isOutput=False)
    vw = nc.declare_dram_parameter("vw", [E, E], F16, isOutput=False)
    vb = nc.declare_dram_parameter("vb", [1, E], F16, isOutput=False)
    conf_rep = nc.declare_dram_parameter("conf_rep", [128, ng], F32, isOutput=False)
    r32 = nc.declare_dram_parameter("r32", [128, 8, 128], F32, isOutput=False)
    m4 = nc.declare_dram_parameter("m4", [128, 4], F32, isOutput=False)
    ones1 = nc.declare_dram_parameter("ones1", [1, 128], F32, isOutput=False)
    ident = nc.declare_dram_parameter("ident", [128, 128], F16, isOutput=False)
    ones_h = nc.declare_dram_parameter("ones_h", [1, 128], F16, isOutput=False)
    out = nc.declare_dram_parameter("out", [ppc, D, 2], F32, isOutput=True)

    from contextlib import ExitStack

    with tile.TileContext(nc) as tc, ExitStack() as es:
            ec = es.enter_context
            cpool = ec(tc.tile_pool(name="const", bufs=1))
            wpool = ec(tc.tile_pool(name="bigw", bufs=1))
            curpool = ec(tc.tile_pool(name="cur", bufs=2))
            qkpool = ec(tc.tile_pool(name="qk", bufs=2))
            qk4pool = ec(tc.tile_pool(name="qk4", bufs=2))
            hpool = ec(tc.tile_pool(name="h", bufs=5))
            prodpool = ec(tc.tile_pool(name="prod", bufs=1))
            smpool = ec(tc.tile_pool(name="sm", bufs=4))
            wdpool = ec(tc.tile_pool(name="wd", bufs=4))
            hsbpool = ec(tc.tile_pool(name="hsb", bufs=4))
            hbpool = ec(tc.tile_pool(name="hbarb", bufs=2))
            htpool = ec(tc.tile_pool(name="hbarT", bufs=8))
            ctxpool = ec(tc.tile_pool(name="ctxs", bufs=2))
            outpool = ec(tc.tile_pool(name="outp", bufs=2))
            ps_sh = ec(tc.tile_pool(name="ps_sh", bufs=2, space="PSUM"))
            ps_rep = ec(tc.tile_pool(name="ps_rep", bufs=2, space="PSUM"))
            ps_hb = ec(tc.tile_pool(name="ps_hb", bufs=1, space="PSUM"))
            del es
            from contextlib import nullcontext
            loop_cm = (
                tc.For_i(0, BENCH_LOOP, 1) if BENCH_LOOP > 0 else nullcontext()
            )
            with loop_cm:
                # ---- constants / weights resident in SBUF ----
                m4_t = cpool.tile([128, 4], F32)
                nc.sync.dma_start(out=m4_t[:], in_=m4[:])
                ones_t = cpool.tile([1, 128], F32R)
                nc.sync.dma_start(out=ones_t[:], in_=ones1[:].bitcast(F32R))
                id_t = cpool.tile([128, 128], F16)
                nc.sync.dma_start(out=id_t[:], in_=ident[:])
                onesh_t = cpool.tile([1, 128], F16)
                nc.sync.dma_start(out=onesh_t[:], in_=ones_h[:])
                bqk_t = cpool.tile([1, E], F32R)
                nc.sync.dma_start(out=bqk_t[:], in_=bqk[:].bitcast(F32R))
                vb_t = cpool.tile([1, E], F16)
                nc.sync.dma_start(out=vb_t[:], in_=vb[:])
                cr_t = cpool.tile([128, ng], F32)
                nc.sync.dma_start(out=cr_t[:], in_=conf_rep[:])
                r32_t = cpool.tile([128, 8, 128], F32R)
                nc.sync.dma_start(out=r32_t[:], in_=r32[:].bitcast(F32R))

                # shares the H-pool slots: freed after phase A so H streaming
                # gets its second buffer back
                c2t_t = hpool.tile([128, 4, 2, ppc], F32R, tag="h")
                nc.sync.dma_start(
                    out=c2t_t[:],
                    in_=cur2t[:].bitcast(F32R).rearrange(
                        "(ka kb p) n -> p ka kb n", p=128, kb=2
                    ),
                )

                wqk_t = wpool.tile([128, 8, E], F32R, tag="bigw")
                nc.sync.dma_start(
                    out=wqk_t[:],
                    in_=wqk[:].bitcast(F32R).rearrange("(k p) e -> p k e", p=128),
                )

                # ---- phase A: Qk = cur2 @ Wqk + bqk, then relayout to [32,4,E]
                # so the per-group replication matmul always reads base partition 0
                qk4s = []
                for b in range(nb):
                    qk_t = qkpool.tile([128, E], F32, tag="qk")
                    for h in range(2):
                        ps = ps_sh.tile([128, 512], F32, tag="mm512")
                        for k in range(8):
                            nc.tensor.matmul(
                                ps[:],
                                lhsT=(
                                    c2t_t[:, k // 2, k % 2, 128 * b : 128 * (b + 1)]
                                ),
                                rhs=(wqk_t[:, k, 512 * h : 512 * (h + 1)]),
                                start=(k == 0),
                                stop=False,
                            )
                        nc.tensor.matmul(
                            ps[:],
                            lhsT=(ones_t[:]),
                            rhs=(bqk_t[:, 512 * h : 512 * (h + 1)]),
                            start=False,
                            stop=True,
                        )
                        nc.scalar.activation(
                            qk_t[:, 512 * h : 512 * (h + 1)], ps[:], AF.Copy
                        )
                    qk4_t = qk4pool.tile([32, 4, E], F32R, tag="qk4")
                    for blk in range(4):
                        nc.gpsimd.dma_start(
                            out=qk4_t[:, blk, :],
                            in_=qk_t[32 * blk : 32 * (blk + 1), :].bitcast(F32R),
                        )
                    qk4s.append(qk4_t)
                    if DEBUG_STOP == "A":
                        nc.sync.dma_start(
                            out=out[:]
                            .rearrange("(bb p) d two -> p bb (d two)", p=128)[:, b],
                            in_=qk_t[:],
                        )

                # vW reuses the Wqk SBUF slot once phase A has consumed it
                vw_t = wpool.tile([128, 8, E], F16, tag="bigw")
                nc.sync.dma_start(
                    out=vw_t[:],
                    in_=vw[:].rearrange("(k p) e -> p k e", p=128),
                )

                # ---- phases B (scores/softmax/hbar) + C (ctx/out) per batch ----
                for b in range(nb if DEBUG_STOP != "A" else 0):
                    cur_t = curpool.tile([128, 2, D], F32, tag="cur")
                    nc.sync.dma_start(
                        out=cur_t[:, 0, :],
                        in_=cur_r[128 * b : 128 * (b + 1), :],
                    )
                    nc.sync.dma_start(
                        out=cur_t[:, 1, :],
                        in_=cur_i[128 * b : 128 * (b + 1), :],
                    )
                    hbar_b = hbpool.tile([128, E], F16)
                    for sb in range(4):  # sub-batches of 8 groups = 32 pairs
                        hts4 = []
                        for half in range(2):
                            h_t = hpool.tile([128, 4, E], F32R, tag="h")
                            p0 = 4 * (32 * b + 8 * sb + 4 * half)
                            nc.sync.dma_start(
                                out=h_t[:, :, 0:D],
                                in_=hist_r[p0 : p0 + 16].bitcast(F32R).rearrange(
                                    "(gl j) s d -> (j s) gl d", j=4
                                ),
                            )
                            nc.sync.dma_start(
                                out=h_t[:, :, D:E],
                                in_=hist_i[p0 : p0 + 16].bitcast(F32R).rearrange(
                                    "(gl j) s d -> (j s) gl d", j=4
                                ),
                            )
                            hts4.append(h_t)

                        scores8 = smpool.tile([128, 8], F32, tag="scores")
                        exp8 = smpool.tile([128, 8], F32, tag="exp")
                        for gl in range(8):
                            g = 32 * b + 8 * sb + gl  # core-local group id
                            qkr = ps_rep.tile([128, E], F32)
                            for h in range(2):
                                nc.tensor.matmul(
                                    qkr[:, 512 * h : 512 * (h + 1)],
                                    lhsT=(r32_t[0:32, gl, :]),
                                    rhs=(qk4s[b][:, sb, 512 * h : 512 * (h + 1)]),
                                    start=True,
                                    stop=True,
                                )
                            # scores: DVE elementwise H*Qk_rep, then ACT
                            # Copy-with-accum folds the conf*scale and reduces
                            # along the free axis (the custom DVE fused reduce op
                            # hangs on this runtime, so standard ops only)
                            prod = prodpool.tile([128, E], F16)
                            nc.vector.tensor_tensor(
                                out=prod[:],
                                in0=hts4[gl // 4][:, gl % 4, :].bitcast(F32),
                                in1=qkr[:],
                                op=AX.mult,
                            )
                            sink = prodpool.tile([128, E], F16, tag="sink")
                            nc.vector.tensor_scalar(
                                sink[:],
                                prod[:],
                                cr_t[:, g : g + 1],
                                None,
                                AX.mult,
                                op1=AX.add,
                                accum_out=scores8[:, gl : gl + 1],
                            )
                        nc.scalar.activation(exp8[:], scores8[:], AF.Exp)

                        dn = ps_sh.tile([4, 8], F32, tag="mm512")
                        nc.tensor.matmul(dn[:], lhsT=m4_t[:], rhs=exp8[:], start=True, stop=True)
                        inv4 = smpool.tile([4, 8], F32, tag="inv")
                        nc.vector.reciprocal(inv4[:], dn[:])

                        if DEBUG_STOP == "B":
                            nc.sync.dma_start(
                                out=out[:]
                                .rearrange("(bb p) d two -> p bb (d two)", p=128)
                                [:, b, 8 * sb : 8 * (sb + 1)],
                                in_=exp8[:],
                            )
                        for gl in range(8 if DEBUG_STOP != "B" else 0):
                            wd = wdpool.tile([128, 4], F32R)
                            nc.vector.tensor_scalar_mul(wd[:], m4_t[:], exp8[:, gl : gl + 1])
                            hb4 = ps_hb.tile([4, E], F32)
                            for h in range(2):
                                nc.tensor.matmul(
                                    hb4[:, 512 * h : 512 * (h + 1)],
                                    lhsT=(wd[:]),
                                    rhs=(
                                        hts4[gl // 4][:, gl % 4, 512 * h : 512 * (h + 1)]
                                    ),
                                    start=True,
                                    stop=True,
                                )
                            hsb4 = hsbpool.tile([4, E], F16)
                            nc.scalar.activation(
                                hsb4[:], hb4[:], AF.Copy, scale=inv4[:, gl : gl + 1]
                            )
                            lp = 4 * (8 * sb + gl)  # batch-local pair of group
                            nc.gpsimd.dma_start(
                                out=hbar_b[lp : lp + 4, :], in_=hsb4[:]
                            )

                    if DEBUG_STOP == "NOCTX":
                        nc.sync.dma_start(
                            out=out[:]
                            .rearrange("(bb p) d two -> p bb (d two)", p=128)[:, b],
                            in_=hbar_b[:],
                        )
                    # transpose hbar [128 pairs, E] -> hbarT chunks [128 e, 128 p]
                    hts = []
                    for c in range(8 if DEBUG_STOP not in ("B", "NOCTX") else 0):
                        tp = ps_sh.tile([128, 128], F16, tag="mm512")
                        nc.tensor.transpose(
                            tp[:], hbar_b[:, 128 * c : 128 * (c + 1)], id_t[:]
                        )
                        ht = htpool.tile([128, 128], F16, tag="hbarT")
                        nc.scalar.activation(ht[:], tp[:], AF.Copy)
                        hts.append(ht)

                    out_t = outpool.tile([128, D, 2], F32)
                    for h2 in range(2 if DEBUG_STOP not in ("B", "NOCTX") else 0):
                        cps = ps_sh.tile([128, 512], F32, tag="mm512")
                        for c in range(8):
                            nc.tensor.matmul(
                                cps[:],
                                lhsT=(hts[c][:]),
                                rhs=(vw_t[:, c, 512 * h2 : 512 * (h2 + 1)]),
                                start=(c == 0),
                                stop=False,
                            )
                        nc.tensor.matmul(
                            cps[:],
                            lhsT=(onesh_t[:]),
                            rhs=(vb_t[:, 512 * h2 : 512 * (h2 + 1)]),
                            start=False,
                            stop=True,
                        )
                        nc.vector.scalar_tensor_tensor(
                            out=out_t[:, :, h2],
                            in0=cps[:],
                            scalar=0.1,
                            in1=cur_t[:, h2, :],
                            op0=AX.mult,
                            op1=AX.add,
                        )
                    if DEBUG_STOP not in ("B", "NOCTX"):
                        nc.sync.dma_start(
                            out=out[:]
                            .rearrange("(b p) d two -> p b d two", p=128)[:, b],
                            in_=out_t[:],
                        )

    # bacc lowering: splits multi-wait instructions (walrus allows only one
    # sync wait per instruction), register allocation, DCE
    nc.compile()
    return nc


_CACHE: dict[int, bass.Bass] = {}


def get_nc(ppc: int) -> bass.Bass:
    if ppc not in _CACHE:
        _CACHE[ppc] = build(ppc)
    return _CACHE[ppc]


def make_const_inputs(ng: int):
    r32_v = np.zeros((8, 32, 128), np.float32)
    for v in range(8):
        for j in range(4):
            r32_v[v, 4 * v + j, 32 * j : 32 * (j + 1)] = 1.0
    # replicated into each 32-partition block so lhsT base can match rhs base
    r32_h = np.ascontiguousarray(np.tile(r32_v.transpose(1, 0, 2), (4, 1, 1)))
    m4_h = np.zeros((128, 4), np.float32)
    for j in range(4):
        m4_h[32 * j : 32 * (j + 1), j] = 1.0
    ones_h = np.ones((1, 128), np.float32)
    id_h = np.eye(128, dtype=np.float16)
    return r32_h, m4_h, ones_h, id_h


def host_prep(hist_real, hist_imag, current_real, current_imag, confidence,
              qW, qb, kW, kb, vW, vb, ppc):
    """Shared host-side folding + per-core input maps."""
    f = lambda x: np.ascontiguousarray(np.asarray(x, dtype=np.float32))
    hist_real, hist_imag = f(hist_real), f(hist_imag)
    current_real, current_imag = f(current_real), f(current_imag)
    confidence = f(confidence)
    qW, qb, kW, kb, vW, vb = f(qW), f(qb), f(kW), f(kb), f(vW), f(vb)

    n_cores = (B * T) // ppc
    wqk_h = np.ascontiguousarray(qW @ kW.T)          # [D2, E]
    bqk_h = (qb @ kW.T).reshape(1, E)                # [1, E]
    vw_h = vW.astype(np.float16)
    vb_h = vb.reshape(1, E).astype(np.float16)
    ng = ppc // 4
    r32_h, m4_h, ones_h, id_h = make_const_inputs(ng)

    hr = hist_real.reshape(B * T, S, D)
    hi = hist_imag.reshape(B * T, S, D)
    cr = current_real.reshape(B * T, D)
    ci = current_imag.reshape(B * T, D)
    cf = confidence.reshape(B * T)

    in_maps = []
    for c in range(n_cores):
        sl = slice(c * ppc, (c + 1) * ppc)
        cur2t_h = np.ascontiguousarray(
            np.concatenate([cr[sl], ci[sl]], axis=1).T
        )  # [D2, ppc]
        c4 = cf[sl].reshape(ng, 4).T * SCALE          # [4, ng]
        conf_rep_h = np.ascontiguousarray(np.repeat(c4, 32, axis=0))  # [128, ng]
        in_maps.append({
            "hist_real": hr[sl],
            "hist_imag": hi[sl],
            "cur_r": cr[sl],
            "cur_i": ci[sl],
            "cur2t": cur2t_h,
            "wqk": wqk_h,
            "bqk": bqk_h,
            "vw": vw_h,
            "vb": vb_h,
            "conf_rep": conf_rep_h,
            "r32": r32_h,
            "m4": m4_h,
            "ones1": ones_h,
            "ident": id_h,
            "ones_h": ones_h.astype(np.float16),
        })
    return in_maps


def kernel(hist_real, hist_imag, current_real, current_imag, confidence,
           qW, qb, kW, kb, vW, vb):
    ppc = PAIRS // N_CORES
    nc = get_nc(ppc)
    in_maps = host_prep(hist_real, hist_imag, current_real, current_imag,
                        confidence, qW, qb, kW, kb, vW, vb, ppc)
    res = run_bass_kernel_spmd(nc, in_maps, list(range(N_CORES))).results
    out = np.concatenate([res[c]["out"] for c in range(N_CORES)], axis=0)
    return out.view(np.complex64)[..., 0].reshape(B, T, D)

